# revision 1
# baseline (speedup 1.0000x reference)
"""Multi-head attention (B=2, S=2048, D=1024, 16 heads) on 8 trn2 cores.

Sharding: tensor-parallel over heads (2 heads = 128 feature dims per core).
Per core, per batch b:
  - Q/K projections computed transposed:  QT/KT [128f, 2048t] = W_c^T @ x^T
  - V projected transposed then PE-transposed back to natural [t, f] layout,
    with a ones-column appended per head (softmax denominator trick)
  - scores computed transposed S^T[k, q] = (KT slice).T @ (QT slice) per head;
    exp on ACT (scale=1/8 folded in); PV matmul lhsT=[V_h|1] gives
    attn^T [64, q] plus the softmax denominator in row 64
  - normalize via reciprocal + gpsimd partition_broadcast + DVE multiply
  - per-batch AllToAll redistributes head-shards -> token-shards (overlaps
    with the other batch's attention)
  - output projection over full 1024 features for this core's 256 tokens of b
Host only slices/transposes inputs and re-assembles the token-shard outputs
(core c owns tokens [256c,256c+256) of each batch).

All matmuls run in float32r (TF32-like, 1 cycle/row at N>=256). DRAM inputs
are declared float32r (identical fp32 bit layout) so plain HWDGE DMAs feed
the PE without cast steps. PSUM budget: score pool 2x[128,1024] (4 banks) +
unified 1-bank pool x4 (proj/transpose/PV/out tiles) = 8 banks, allowing all
phases to overlap.
"""

import numpy as np

import concourse.bacc as bacc
import concourse.mybir as mybir
import concourse.tile as tile
from concourse.bass_utils import run_bass_kernel_spmd
from concourse.masks import make_identity

F32 = mybir.dt.float32
F32R = mybir.dt.float32r
AF = mybir.ActivationFunctionType

NCORES = 8
B, S, D = 2, 2048, 1024
T = B * S            # 4096 flattened tokens
HD = 64              # head dim
FPC = 128            # feature dims per core (2 heads)
TPB = S // NCORES    # 256 tokens per core per batch for the output projection
NDT = D // 128       # 8 contraction tiles of 128
QBLK = 512           # q tile (free dim) in attention
NKT = S // 128       # 16 k tiles per (b, h)
KGRP = 2             # k-tiles per exp() activation instruction


def build_nc(debug=False, repeat=1, use_cc=True):
    nc = bacc.Bacc(trn_type="TRN2", num_devices=NCORES)

    xqT = nc.dram_tensor("xqT", [D, T], F32R, kind="ExternalInput")
    xkT = nc.dram_tensor("xkT", [D, T], F32R, kind="ExternalInput")
    xvT = nc.dram_tensor("xvT", [D, T], F32R, kind="ExternalInput")
    wq = nc.dram_tensor("wq", [D, FPC], F32R, kind="ExternalInput")
    wk = nc.dram_tensor("wk", [D, FPC], F32R, kind="ExternalInput")
    wv = nc.dram_tensor("wv", [D, FPC], F32R, kind="ExternalInput")
    wo = nc.dram_tensor("wo", [D, D], F32R, kind="ExternalInput")
    bq = nc.dram_tensor("bq", [FPC], F32, kind="ExternalInput")
    bk = nc.dram_tensor("bk", [FPC], F32, kind="ExternalInput")
    bv = nc.dram_tensor("bv", [FPC], F32, kind="ExternalInput")
    bo = nc.dram_tensor("bo", [D], F32R, kind="ExternalInput")
    # rows [b*256, b*256+256) = tokens [b*2048+256c, +256) of batch b
    y = nc.dram_tensor("y", [B * TPB, D], F32, kind="ExternalOutput")
    if debug:
        dbg = {
            "dbg_qt": nc.dram_tensor("dbg_qt", [128, T], F32R, kind="ExternalOutput"),
            "dbg_kt": nc.dram_tensor("dbg_kt", [128, T], F32R, kind="ExternalOutput"),
            "dbg_v": nc.dram_tensor("dbg_v", [128, NKT * B * 130], F32R,
                                    kind="ExternalOutput"),
        }

    with tile.TileContext(nc) as tc:
        with (
            tc.tile_pool(name="persist", bufs=1) as persist,
            tc.tile_pool(name="dram", bufs=1, space="DRAM") as dram,
            tc.tile_pool(name="xs", bufs=10) as xs_pool,
            tc.tile_pool(name="vt_tmp", bufs=2) as vt_tmp_pool,
            tc.tile_pool(name="es", bufs=6) as es_pool,
            tc.tile_pool(name="bc", bufs=2) as bc_pool,
            tc.tile_pool(name="rt", bufs=2) as rt_pool,
            tc.tile_pool(name="afl", bufs=1) as afl_pool,
            tc.tile_pool(name="ysb", bufs=2) as ysb_pool,
            tc.tile_pool(name="s_ps", bufs=2, space="PSUM") as s_ps_pool,
            tc.tile_pool(name="uni_ps", bufs=4, space="PSUM") as uni_ps_pool,
        ):
            # ---- resident weights / constants ----
            wq_sb = persist.tile([128, NDT, FPC], F32R)
            wk_sb = persist.tile([128, NDT, FPC], F32R)
            wv_sb = persist.tile([128, NDT, FPC], F32R)
            for w_dram, w_sb in ((wk, wk_sb), (wq, wq_sb), (wv, wv_sb)):
                nc.sync.dma_start(
                    w_sb[:], w_dram.rearrange("(dt p) f -> p dt f", p=128)
                )
            wo_sb = persist.tile([128, NDT, D], F32R)
            bq_sb = persist.tile([128, 1], F32)
            bk_sb = persist.tile([128, 1], F32)
            bv_sb = persist.tile([64, 2], F32)   # col h = bias for head h
            nc.sync.dma_start(bq_sb[:], bq[:, None])
            nc.sync.dma_start(bk_sb[:], bk[:, None])
            nc.sync.dma_start(bv_sb[:], bv.rearrange("(h p) -> p h", p=64))
            bo_sb = persist.tile([1, D], F32R)
            nc.sync.dma_start(bo_sb[:], bo[None, :])
            const_f32 = persist.tile([128, 128], F32)
            nc.vector.memset(const_f32[:], 1.0)
            ones_sb = persist.tile([1, 128], F32R)
            nc.vector.tensor_copy(ones_sb[:], const_f32[0:1, :])
            ident_f32 = persist.tile([128, 128], F32)
            make_identity(nc, ident_f32[:])
            ident = persist.tile([128, 128], F32R)
            nc.vector.tensor_copy(ident[:], ident_f32[:])

            # ---- persistent activations ----
            qt_sb = persist.tile([128, T], F32R)   # QT: [feat, tok]
            kt_sb = persist.tile([128, T], F32R)   # KT
            v_sb = persist.tile([128, NKT * B, 130], F32R)  # [k-in-tile, kt, VA|1|VB|1]
            # per-head attn^T; rows 0:64 = attn values, row 64 = softmax denom
            attn_h = [persist.tile([65, S], F32R, name=f"attnh{h}") for h in range(2)]
            

            # ones columns of V (heads A and B)
            nc.vector.tensor_copy(
                v_sb[:, :, 64:65].rearrange("p a b -> p (a b)"), const_f32[:, 0:32]
            )
            nc.vector.tensor_copy(
                v_sb[:, :, 129:130].rearrange("p a b -> p (a b)"), const_f32[:, 0:32]
            )

            kgrps = []
            k0 = 0
            while k0 < NKT:
                kgrps.append((k0, min(KGRP, NKT - k0)))
                k0 += KGRP

            def proj_chunk(b, kind, tb):
                """One 512-token t_blk of the K/Q/V projection for batch b."""
                x_dram, w_sb, bias, dst = {
                    "K": (xkT, wk_sb, bk_sb, kt_sb),
                    "Q": (xqT, wq_sb, bq_sb, qt_sb),
                    "V": (xvT, wv_sb, None, None),
                }[kind]
                tlo = b * S + tb * QBLK
                ps = uni_ps_pool.tile([128, QBLK], F32, tag="uni")
                for dt_i in range(NDT):
                    xt = xs_pool.tile([128, QBLK], F32R, tag="xt")
                    nc.sync.dma_start(
                        xt[:],
                        x_dram[128 * dt_i:128 * (dt_i + 1), tlo:tlo + QBLK],
                    )
                    nc.tensor.matmul(
                        ps[:], lhsT=w_sb[:, dt_i, :], rhs=xt[:],
                        start=(dt_i == 0), stop=(dt_i == NDT - 1),
                    )
                if kind != "V":
                    nc.vector.tensor_scalar_add(
                        dst[:, tlo:tlo + QBLK], ps[:], bias[:]
                    )
                    return
                # V: PE-transpose into natural [tok, feat] layout
                vt_tmp = vt_tmp_pool.tile([128, QBLK], F32R, tag="vt_tmp")
                nc.vector.tensor_copy(vt_tmp[:], ps[:])
                for j in range(QBLK // 128):
                    kt_i = (b * 4 + tb) * 4 + j
                    tp = uni_ps_pool.tile([128, 128], F32R, tag="uni")
                    nc.tensor.transpose(
                        tp[:], vt_tmp[:, 128 * j:128 * (j + 1)], ident[:]
                    )
                    # cols 0:64 -> V 0:64 (head A); 64:128 -> 65:129 (head B)
                    nc.vector.tensor_copy(
                        v_sb[:, kt_i, :].rearrange("p (h x) -> p h x", h=2)[
                            :, :, 0:64
                        ],
                        tp[:].rearrange("p (h x) -> p h x", h=2),
                    )

            def attn_chunk(b, h, qb, a2a_dst, group_feed=None):
                hs = 64 * h
                qlo = b * S + qb * QBLK
                pv = uni_ps_pool.tile([65, QBLK], F32, tag="uni")
                for gi, (k0, klen) in enumerate(kgrps):
                    if group_feed is not None and gi in group_feed:
                        group_feed[gi]()
                    sp = s_ps_pool.tile([128, KGRP * QBLK], F32, tag="sp")
                    for i in range(klen):
                        klo = b * S + (k0 + i) * 128
                        nc.tensor.matmul(
                            sp[:, QBLK * i:QBLK * (i + 1)],
                            lhsT=kt_sb[hs:hs + 64, klo:klo + 128],
                            rhs=qt_sb[hs:hs + 64, qlo:qlo + QBLK],
                            start=True, stop=True,
                        )
                    es = es_pool.tile([128, KGRP * QBLK], F32R, tag="es")
                    nc.scalar.activation(
                        es[:, :QBLK * klen], sp[:, :QBLK * klen],
                        AF.Exp, scale=0.125,
                    )
                    for i in range(klen):
                        kt_i = b * NKT + k0 + i
                        nc.tensor.matmul(
                            pv[:],
                            lhsT=v_sb[:, kt_i, 65 * h:65 * (h + 1)],
                            rhs=es[:, QBLK * i:QBLK * (i + 1)],
                            start=(k0 + i == 0), stop=(k0 + i == NKT - 1),
                        )

                def flush():
                    # evacuate; then normalize this q-slice and stage it into
                    # the a2a input buffer (emitted late so the pv-stop wait
                    # does not head-block DVE before feed-chunk bias adds)
                    qsl = slice(qb * QBLK, (qb + 1) * QBLK)
                    nc.vector.tensor_copy(attn_h[h][:, qsl], pv[:])
                    rt = rt_pool.tile([1, QBLK], F32R, tag="rt")
                    nc.sync.dma_start(rt[:], attn_h[h][64:65, qsl])
                    with nc.allow_low_precision(reason="f32r softmax denom"):
                        nc.vector.reciprocal(rt[:], rt[:])
                    bcast = bc_pool.tile([64, QBLK], F32R, tag="bcast")
                    nc.gpsimd.partition_broadcast(bcast[:], rt[:])
                    nc.vector.tensor_tensor(
                        attn_h[h][0:64, qsl], attn_h[h][0:64, qsl], bcast[:],
                        mybir.AluOpType.mult,
                    )
                    nc.vector.tensor_scalar_add(
                        attn_h[h][0:64, qsl], attn_h[h][0:64, qsl],
                        bv_sb[:, h:h + 1],
                    )
                    # stage into the a2a input: nsh shards cover this q-slice
                    buf, sh0, nsh = a2a_dst(qb)
                    nc.sync.dma_start(
                        buf.rearrange("r (p t) -> p r t", p=128)[
                            64 * h:64 * (h + 1), sh0:sh0 + nsh, :
                        ],
                        attn_h[h][0:64, qsl].rearrange("p (r t) -> p r t", r=nsh),
                    )

                return flush

            def a2a_start(a2a_in, a2a_out, tw):
                # inputs already staged incrementally by the chunk flushes
                if use_cc:
                    nc.gpsimd.collective_compute(
                        "AllToAll",
                        mybir.AluOpType.bypass,
                        ins=[a2a_in[:]],
                        outs=[a2a_out[:]],
                        replica_groups=[list(range(NCORES))],
                    )
                else:
                    nc.sync.dma_start(a2a_out[:], a2a_in[:])
                afull = afl_pool.tile([128, NCORES, TPB], F32R, tag="afull")
                nc.sync.dma_start(
                    afull[:, :, :tw], a2a_out.rearrange("r (p t) -> p r t", p=128)
                )
                return afull

            def out_chunk(afull, tj, yrow):
                y_sb = ysb_pool.tile([128, D], F32, tag="ysb")
                for n in range(D // QBLK):
                    yp = uni_ps_pool.tile([128, QBLK], F32, tag="uni")
                    nc.tensor.matmul(
                        yp[:], lhsT=ones_sb[:, 0:128],
                        rhs=bo_sb[:, QBLK * n:QBLK * (n + 1)],
                        start=True, stop=False,
                    )
                    for fi in range(NDT):
                        nc.tensor.matmul(
                            yp[:],
                            lhsT=afull[:, fi, 128 * tj:128 * (tj + 1)],
                            rhs=wo_sb[:, fi, QBLK * n:QBLK * (n + 1)],
                            start=False, stop=(fi == NDT - 1),
                        )
                    nc.vector.tensor_copy(y_sb[:, QBLK * n:QBLK * (n + 1)], yp[:])
                nc.sync.dma_start(y[yrow:yrow + 128, :], y_sb[:])

            def load_wo():
                nc.sync.dma_start(
                    wo_sb[:], wo.rearrange("(ft p) e -> p ft e", p=128)
                )

            for rep in range(repeat):
                # b0: one full-batch a2a (256-token shards);
                # b1: two half-batch a2as (128-token shards) so the second
                # collective + output projection overlap attention
                ab0 = (dram.tile([NCORES, FPC * TPB], F32R, name=f"a0i_{rep}"),
                       dram.tile([NCORES, FPC * TPB], F32R, name=f"a0o_{rep}"))
                ab1 = [(dram.tile([NCORES, FPC * 128], F32R, name=f"a1i{i}_{rep}"),
                        dram.tile([NCORES, FPC * 128], F32R, name=f"a1o{i}_{rep}"))
                       for i in range(2)]

                def a2a_dst0(qb):
                    return ab0[0], 2 * qb, 2

                def a2a_dst1(qb):
                    return ab1[qb // 2][0], 4 * (qb % 2), 4

                # batch-0 lead-in: Q.tb0 + all K; V tiles are emitted inside
                # attn chunk 0 right before the PV groups that need them, so
                # the first chunk's score matmuls aren't head-blocked by late
                # V-projection matmuls in the PE stream
                proj_chunk(0, "Q", 0)
                for tb in range(4):
                    proj_chunk(0, "K", tb)
                gfeed = {2 * tb: (lambda tb=tb: proj_chunk(0, "V", tb))
                         for tb in range(4)}
                feed = (
                    [("Q", 0, tb) for tb in (1, 2, 3)]
                    + [("Q", 1, 0)]
                    + [x for tb in range(4) for x in (("K", 1, tb), ("V", 1, tb))]
                    + [("Q", 1, tb) for tb in (1, 2, 3)]
                    + ([("WO",)] if rep == 0 else [])
                )
                achunks = [(h, qb) for qb in range(4) for h in range(2)]
                fi_ = 0

                def emit_feed(n):
                    nonlocal fi_
                    for _ in range(n):
                        if fi_ < len(feed):
                            f = feed[fi_]
                            fi_ += 1
                            if f[0] == "WO":
                                load_wo()
                            else:
                                proj_chunk(f[1], f[0], f[2])

                pend = attn_chunk(0, *achunks[0], a2a_dst0, group_feed=gfeed)
                for ch in achunks[1:]:
                    emit_feed(2)
                    pend()
                    pend = attn_chunk(0, *ch, a2a_dst0)
                emit_feed(len(feed))
                pend()
                af0 = a2a_start(*ab0, TPB)
                # attn(1) with out(0)/out(1a) interleaved; out chunks are
                # emitted after the following attn chunk's matmuls so their
                # (late-arriving) afull dependencies never head-block the
                # PE stream ahead of attention work
                pend = attn_chunk(1, 0, 0, a2a_dst1)
                for i, (h, qb) in enumerate([(1, 0), (0, 1), (1, 1), (0, 2),
                                             (1, 2), (0, 3), (1, 3)]):
                    pend()
                    if i == 3:
                        # qb0-1 flushes done -> b1 first-half a2a can fire
                        af1a = a2a_start(*ab1[0], 128)
                    pend = attn_chunk(1, h, qb, a2a_dst1)
                    if i == 4:
                        out_chunk(af0, 0, 0)
                    elif i == 5:
                        out_chunk(af0, 1, 128)
                        out_chunk(af1a, 0, 256)
                pend()
                af1b = a2a_start(*ab1[1], 128)
                out_chunk(af1b, 0, 384)

            if debug:
                nc.sync.dma_start(dbg["dbg_qt"][:], qt_sb[:])
                nc.sync.dma_start(dbg["dbg_kt"][:], kt_sb[:])
                nc.sync.dma_start(
                    dbg["dbg_v"].rearrange("p (k c) -> p k c", c=130), v_sb[:]
                )

    nc.finalize()
    return nc


_NC_CACHE = None


def kernel(query_inp, key_inp, value_inp, Wq, bq, Wk, bk, Wv, bv, Wo, bo):
    y, _ = _run(query_inp, key_inp, value_inp, Wq, bq, Wk, bk, Wv, bv, Wo, bo)
    return y


def _make_in_maps(query_inp, key_inp, value_inp, Wq, bq, Wk, bk, Wv, bv, Wo, bo):
    xqT = np.ascontiguousarray(np.asarray(query_inp, np.float32).reshape(T, D).T)
    xkT = np.ascontiguousarray(np.asarray(key_inp, np.float32).reshape(T, D).T)
    xvT = np.ascontiguousarray(np.asarray(value_inp, np.float32).reshape(T, D).T)
    Wq = np.asarray(Wq, dtype=np.float32)
    Wk = np.asarray(Wk, dtype=np.float32)
    Wv = np.asarray(Wv, dtype=np.float32)
    Wo = np.ascontiguousarray(np.asarray(Wo, dtype=np.float32))
    in_maps = []
    for c in range(NCORES):
        sl = slice(FPC * c, FPC * (c + 1))
        in_maps.append({
            "xqT": xqT, "xkT": xkT, "xvT": xvT,
            "wq": np.ascontiguousarray(Wq[:, sl]),
            "wk": np.ascontiguousarray(Wk[:, sl]),
            "wv": np.ascontiguousarray(Wv[:, sl]),
            "wo": Wo,
            "bq": np.ascontiguousarray(np.asarray(bq, np.float32)[sl]),
            "bk": np.ascontiguousarray(np.asarray(bk, np.float32)[sl]),
            "bv": np.ascontiguousarray(np.asarray(bv, np.float32)[sl]),
            "bo": np.ascontiguousarray(np.asarray(bo, np.float32)),
        })
    return in_maps


def _assemble(results):
    """y rows per core c: [0,256) = b0 tokens [256c,+256);
    [256,384) = b1 tokens [128c,+128); [384,512) = b1 tokens [1024+128c,+128)."""
    y_full = np.empty((T, D), np.float32)
    for c in range(NCORES):
        yc = results[c]["y"]
        y_full[TPB * c: TPB * (c + 1)] = yc[0:TPB]
        y_full[S + 128 * c: S + 128 * (c + 1)] = yc[TPB:TPB + 128]
        y_full[S + 1024 + 128 * c: S + 1024 + 128 * (c + 1)] = yc[TPB + 128:]
    return y_full.reshape(B, S, D)


def _run(query_inp, key_inp, value_inp, Wq, bq, Wk, bk, Wv, bv, Wo, bo,
         **run_kwargs):
    global _NC_CACHE
    if _NC_CACHE is None:
        _NC_CACHE = build_nc()
    nc = _NC_CACHE
    in_maps = _make_in_maps(query_inp, key_inp, value_inp,
                            Wq, bq, Wk, bk, Wv, bv, Wo, bo)
    res = run_bass_kernel_spmd(nc, in_maps, core_ids=list(range(NCORES)),
                               **run_kwargs)
    return _assemble(res.results), res



# revision 16
# speedup vs baseline: 3.9280x; 3.9280x over previous
"""Multi-head attention (B=2, S=2048, D=1024, 16 heads) on 8 trn2 cores.

Sharding: tensor-parallel over heads (2 heads = 128 feature dims per core).
v2: all-bf16 data path (host-cast inputs/weights), batched x-tile DMAs
(one [128, 8, 512] load per projection chunk), fused flush (DVE reads the
softmax denominator straight out of PSUM, normalize writes bf16), bf16
AllToAll payloads, direct PSUM->DRAM output stores, and a schedule that
starts the batch-0 collective as early as possible while interleaving two
batch-1 attention chunks into the batch-0 phase to balance the ACT (exp)
engine load across the whole kernel.

Per core, per batch b:
  - Q/K projections computed transposed:  QT/KT [128f, 2048t] = W_c^T @ x^T
  - V projected transposed then PE-transposed back to natural [t, f] layout,
    with a ones-column appended per head (softmax denominator trick)
  - scores computed transposed S^T[k, q] = (KT slice).T @ (QT slice) per head;
    exp on ACT (scale=1/8 folded in) -> bf16; PV matmul lhsT=[V_h|1] gives
    attn^T [64, q] plus the softmax denominator in row 64
  - normalize: DVE copies denom row from PSUM, reciprocal, gpsimd
    partition_broadcast, DVE multiply (PSUM x bcast -> bf16 SBUF)
  - per-batch AllToAll redistributes head-shards -> token-shards
  - output projection over full 1024 features for this core's 256 tokens of b
Host only slices/casts/transposes inputs and re-assembles the token-shard
outputs (core c owns tokens [256c,256c+256) of each batch).
"""

import numpy as np
import ml_dtypes

import concourse.bacc as bacc
import concourse.mybir as mybir
import concourse.tile as tile
from concourse.bass_utils import run_bass_kernel_spmd
from concourse.masks import make_identity

F32 = mybir.dt.float32
BF16 = mybir.dt.bfloat16
AF = mybir.ActivationFunctionType

NCORES = 8
B, S, D = 2, 2048, 1024
T = B * S            # 4096 flattened tokens
HD = 64              # head dim
FPC = 128            # feature dims per core (2 heads)
TPB = S // NCORES    # 256 tokens per core per batch for the output projection
NDT = D // 128       # 8 contraction tiles of 128
QBLK = 512           # q tile (free dim) in attention
NKT = S // 128       # 16 k tiles per (b, h)
KGRP = 2             # k-tiles per exp() activation instruction


def build_nc(repeat=1, use_cc=True):
    nc = bacc.Bacc(trn_type="TRN2", num_devices=NCORES)

    xqT = nc.dram_tensor("xqT", [D, T], BF16, kind="ExternalInput")
    xkT = nc.dram_tensor("xkT", [D, T], BF16, kind="ExternalInput")
    xvT = nc.dram_tensor("xvT", [D, T], BF16, kind="ExternalInput")
    wq = nc.dram_tensor("wq", [D, FPC], BF16, kind="ExternalInput")
    wk = nc.dram_tensor("wk", [D, FPC], BF16, kind="ExternalInput")
    wv = nc.dram_tensor("wv", [D, FPC], BF16, kind="ExternalInput")
    wo = nc.dram_tensor("wo", [D, D], BF16, kind="ExternalInput")
    bq = nc.dram_tensor("bq", [FPC], F32, kind="ExternalInput")
    bk = nc.dram_tensor("bk", [FPC], F32, kind="ExternalInput")
    bv = nc.dram_tensor("bv", [FPC], F32, kind="ExternalInput")
    bo = nc.dram_tensor("bo", [D], BF16, kind="ExternalInput")
    # rows [b*256, b*256+256) = tokens [b*2048+256c, +256) of batch b
    y = nc.dram_tensor("y", [B * TPB, D], F32, kind="ExternalOutput")

    with tile.TileContext(nc) as tc:
        with (
            tc.tile_pool(name="persist", bufs=1) as persist,
            tc.tile_pool(name="dram", bufs=1, space="DRAM") as dram,
            tc.tile_pool(name="xs", bufs=3) as xs_pool,
            tc.tile_pool(name="vt_tmp", bufs=2) as vt_tmp_pool,
            tc.tile_pool(name="es", bufs=6) as es_pool,
            tc.tile_pool(name="den", bufs=4) as den_pool,
            tc.tile_pool(name="bc", bufs=2) as bc_pool,
            tc.tile_pool(name="afl", bufs=2) as afl_pool,
            tc.tile_pool(name="ysb", bufs=2) as ysb_pool,
            tc.tile_pool(name="s_ps", bufs=2, space="PSUM") as s_ps_pool,
            tc.tile_pool(name="uni_ps", bufs=4, space="PSUM") as uni_ps_pool,
        ):
            # ---- resident weights / constants ----
            wq_sb = persist.tile([128, NDT, FPC], BF16)
            wk_sb = persist.tile([128, NDT, FPC], BF16)
            wv_sb = persist.tile([128, NDT, FPC], BF16)
            for w_dram, w_sb in ((wk, wk_sb), (wq, wq_sb), (wv, wv_sb)):
                nc.sync.dma_start(
                    w_sb[:], w_dram.rearrange("(dt p) f -> p dt f", p=128)
                )
            wo_sb = persist.tile([128, NDT, D], BF16)
            bq_sb = persist.tile([128, 1], F32)
            bk_sb = persist.tile([128, 1], F32)
            bv_sb = persist.tile([64, 2], F32)   # col h = bias for head h
            nc.sync.dma_start(bq_sb[:], bq[:, None])
            nc.sync.dma_start(bk_sb[:], bk[:, None])
            nc.sync.dma_start(bv_sb[:], bv.rearrange("(h p) -> p h", p=64))
            bo_sb = persist.tile([1, D], BF16)
            nc.sync.dma_start(bo_sb[:], bo[None, :])
            bo_bc = persist.tile([128, D], BF16)
            nc.gpsimd.partition_broadcast(bo_bc[:], bo_sb[:])
            const_f32 = persist.tile([128, 128], F32)
            nc.vector.memset(const_f32[:], 1.0)
            const_bf = persist.tile([128, 128], BF16)
            nc.vector.tensor_copy(const_bf[:], const_f32[:])
            ones_sb = persist.tile([1, 128], BF16)
            nc.vector.tensor_copy(ones_sb[:], const_f32[0:1, :])
            ident_f32 = persist.tile([128, 128], F32)
            make_identity(nc, ident_f32[:])
            ident = persist.tile([128, 128], BF16)
            nc.vector.tensor_copy(ident[:], ident_f32[:])

            # ---- persistent activations ----
            qt_sb = persist.tile([128, T], BF16)   # QT: [feat, tok]
            kt_sb = persist.tile([128, T], BF16)   # KT
            v_sb = persist.tile([128, NKT * B, 130], BF16)  # [k, kt, VA|1|VB|1]
            # per-head normalized attn^T (bf16) for a2a staging
            attn_bf = [persist.tile([64, S], BF16, name=f"attnbf{h}")
                       for h in range(2)]

            # ones columns of V (heads A and B)
            nc.vector.tensor_copy(
                v_sb[:, :, 64:65].rearrange("p a b -> p (a b)"), const_bf[:, 0:32]
            )
            nc.vector.tensor_copy(
                v_sb[:, :, 129:130].rearrange("p a b -> p (a b)"), const_bf[:, 0:32]
            )

            kgrps = []
            k0 = 0
            while k0 < NKT:
                kgrps.append((k0, min(KGRP, NKT - k0)))
                k0 += KGRP

            def proj_chunk(b, kind, tb, split=1):
                """One 512-token t_blk of the K/Q/V projection for batch b."""
                x_dram, w_sb, bias, dst = {
                    "K": (xkT, wk_sb, bk_sb, kt_sb),
                    "Q": (xqT, wq_sb, bq_sb, qt_sb),
                    "V": (xvT, wv_sb, None, None),
                }[kind]
                tlo = b * S + tb * QBLK
                ps = uni_ps_pool.tile([128, QBLK], F32, tag="uni")
                dtg = NDT // split
                for s in range(split):
                    xt = xs_pool.tile([128, dtg, QBLK], BF16, tag="xt")
                    nc.sync.dma_start(
                        xt[:],
                        x_dram[128 * dtg * s:128 * dtg * (s + 1),
                               tlo:tlo + QBLK].rearrange(
                            "(dt p) t -> p dt t", p=128),
                    )
                    for i in range(dtg):
                        dt_i = dtg * s + i
                        nc.tensor.matmul(
                            ps[:], lhsT=w_sb[:, dt_i, :], rhs=xt[:, i, :],
                            start=(dt_i == 0), stop=(dt_i == NDT - 1),
                        )
                if kind != "V":
                    nc.vector.tensor_scalar_add(
                        dst[:, tlo:tlo + QBLK], ps[:], bias[:]
                    )
                    return
                # V: PE-transpose into natural [tok, feat] layout
                vt_tmp = vt_tmp_pool.tile([128, QBLK], BF16, tag="vt_tmp")
                nc.vector.tensor_copy(vt_tmp[:], ps[:])
                for j in range(QBLK // 128):
                    kt_i = (b * 4 + tb) * 4 + j
                    tp = uni_ps_pool.tile([128, 128], BF16, tag="uni")
                    nc.tensor.transpose(
                        tp[:], vt_tmp[:, 128 * j:128 * (j + 1)], ident[:]
                    )
                    # cols 0:64 -> V 0:64 (head A); 64:128 -> 65:129 (head B)
                    nc.vector.tensor_copy(
                        v_sb[:, kt_i, :].rearrange("p (h x) -> p h x", h=2)[
                            :, :, 0:64
                        ],
                        tp[:].rearrange("p (h x) -> p h x", h=2),
                    )

            def attn_chunk(b, h, qb, a2a_dst, group_feed=None):
                hs = 64 * h
                qlo = b * S + qb * QBLK
                pv = uni_ps_pool.tile([65, QBLK], F32, tag="uni")
                for gi, (k0, klen) in enumerate(kgrps):
                    if group_feed is not None and gi in group_feed:
                        group_feed[gi]()
                    sp = s_ps_pool.tile([128, KGRP * QBLK], F32, tag="sp")
                    for i in range(klen):
                        klo = b * S + (k0 + i) * 128
                        nc.tensor.matmul(
                            sp[:, QBLK * i:QBLK * (i + 1)],
                            lhsT=kt_sb[hs:hs + 64, klo:klo + 128],
                            rhs=qt_sb[hs:hs + 64, qlo:qlo + QBLK],
                            start=True, stop=True,
                        )
                    es = es_pool.tile([128, KGRP * QBLK], BF16, tag="es")
                    nc.scalar.activation(
                        es[:, :QBLK * klen], sp[:, :QBLK * klen],
                        AF.Exp, scale=0.125,
                    )
                    for i in range(klen):
                        kt_i = b * NKT + k0 + i
                        nc.tensor.matmul(
                            pv[:],
                            lhsT=v_sb[:, kt_i, 65 * h:65 * (h + 1)],
                            rhs=es[:, QBLK * i:QBLK * (i + 1)],
                            start=(k0 + i == 0), stop=(k0 + i == NKT - 1),
                        )

                def flush():
                    qsl = slice(qb * QBLK, (qb + 1) * QBLK)
                    den = den_pool.tile([1, QBLK], F32, tag="den")
                    nc.vector.tensor_copy(den[:], pv[64:65, :])
                    with nc.allow_low_precision(reason="softmax denom recip"):
                        nc.vector.reciprocal(den[:], den[:])
                    bcast = bc_pool.tile([64, QBLK], F32, tag="bcast")
                    nc.gpsimd.partition_broadcast(bcast[:], den[:])
                    nc.vector.tensor_tensor(
                        attn_bf[h][:, qsl], pv[0:64, :], bcast[:],
                        mybir.AluOpType.mult,
                    )
                    nc.vector.tensor_scalar_add(
                        attn_bf[h][:, qsl], attn_bf[h][:, qsl],
                        bv_sb[:, h:h + 1],
                    )
                    # stage into the a2a input: nsh shards cover this q-slice
                    buf, sh0, nsh = a2a_dst(qb)
                    nc.sync.dma_start(
                        buf.rearrange("r (p t) -> p r t", p=128)[
                            64 * h:64 * (h + 1), sh0:sh0 + nsh, :
                        ],
                        attn_bf[h][:, qsl].rearrange("p (r t) -> p r t", r=nsh),
                    )

                return flush

            def a2a_start(a2a_in, a2a_out, tw):
                # inputs already staged incrementally by the chunk flushes
                if use_cc:
                    nc.gpsimd.collective_compute(
                        "AllToAll",
                        mybir.AluOpType.bypass,
                        ins=[a2a_in[:]],
                        outs=[a2a_out[:]],
                        replica_groups=[list(range(NCORES))],
                    )
                else:
                    nc.sync.dma_start(a2a_out[:], a2a_in[:])
                afull = afl_pool.tile([128, NCORES, TPB], BF16, tag="afull")
                nc.sync.dma_start(
                    afull[:, :, :tw], a2a_out.rearrange("r (p t) -> p r t", p=128)
                )
                return afull

            def out_chunk(afull, tj, yrow):
                y_sb = ysb_pool.tile([128, D], F32, tag="ysb")
                for n in range(D // QBLK):
                    yp = uni_ps_pool.tile([128, QBLK], F32, tag="uni")
                    for fi in range(NDT):
                        nc.tensor.matmul(
                            yp[:],
                            lhsT=afull[:, fi, 128 * tj:128 * (tj + 1)],
                            rhs=wo_sb[:, fi, QBLK * n:QBLK * (n + 1)],
                            start=(fi == 0), stop=(fi == NDT - 1),
                        )
                    nc.vector.tensor_tensor(
                        y_sb[:, QBLK * n:QBLK * (n + 1)], yp[:],
                        bo_bc[:, QBLK * n:QBLK * (n + 1)], mybir.AluOpType.add,
                    )
                nc.sync.dma_start(y[yrow:yrow + 128, :], y_sb[:])

            def load_wo():
                nc.sync.dma_start(
                    wo_sb[:], wo.rearrange("(ft p) e -> p ft e", p=128)
                )

            for rep in range(repeat):
                # one full-batch a2a (256-token shards) per batch
                ab0 = (dram.tile([NCORES, FPC * TPB], BF16, name=f"a0i_{rep}"),
                       dram.tile([NCORES, FPC * TPB], BF16, name=f"a0o_{rep}"))
                ab1 = (dram.tile([NCORES, FPC * TPB], BF16, name=f"a1i_{rep}"),
                       dram.tile([NCORES, FPC * TPB], BF16, name=f"a1o_{rep}"))

                def a2a_dst0(qb):
                    return ab0[0], 2 * qb, 2

                def a2a_dst1(qb):
                    return ab1[0], 2 * qb, 2

                # batch-0 lead-in: only Q.tb0 + K.tb0; later K and V chunks
                # are emitted inside attn chunk 0 just before the score/PV
                # groups that consume them (group g uses k-tiles 2g, 2g+1;
                # proj chunk tb covers tiles 4tb..4tb+3).
                proj_chunk(0, "Q", 0, split=2)
                proj_chunk(0, "K", 0, split=2)
                def _gf(*specs):
                    def emit():
                        for kind, tb in specs:
                            proj_chunk(0, kind, tb)
                    return emit

                gfeed = {
                    0: _gf(("V", 0), ("K", 1)),
                    1: _gf(("V", 1)),
                    2: _gf(("K", 2)),
                    3: _gf(("V", 2)),
                    4: _gf(("K", 3)),
                    5: _gf(("V", 3)),
                }
                feed = (
                    [("Q", 0, tb) for tb in (1, 2, 3)]
                    + [x for tb in range(4) for x in (("K", 1, tb), ("V", 1, tb))]
                    + [("Q", 1, 0)]
                    + [("Q", 1, 1), ("Q", 1, 2), ("Q", 1, 3)]
                    + ([("WO",)] if rep == 0 else [])
                )
                fi_ = 0

                def emit_feed(n):
                    nonlocal fi_
                    for _ in range(n):
                        if fi_ < len(feed):
                            f = feed[fi_]
                            fi_ += 1
                            if f[0] == "WO":
                                load_wo()
                            else:
                                proj_chunk(f[1], f[0], f[2])

                # phase 1: b0 chunks 0-5, then two early b1 qb0 chunks (their
                # exp load lands in the b0 window), then b0 chunks 6-7.
                plan0 = [(0, h, qb) for qb in range(4) for h in range(2)]
                phase1 = plan0[:6] + [(1, 0, 0), (1, 1, 0)] + plan0[6:]

                pend = attn_chunk(0, *phase1[0][1:], a2a_dst0, group_feed=gfeed)
                for (b, h, qb) in phase1[1:]:
                    emit_feed(2)
                    pend()
                    dst = a2a_dst0 if b == 0 else a2a_dst1
                    pend = attn_chunk(b, h, qb, dst)
                pend()
                emit_feed(len(feed))
                af0 = a2a_start(*ab0, TPB)

                # phase 2: remaining b1 chunks; out chunks for batch 0
                # interleave once af0 is available; one b1 a2a at the end.
                phase2 = [(0, 1), (1, 1), (0, 2), (1, 2), (0, 3), (1, 3)]
                pend = attn_chunk(1, *phase2[0], a2a_dst1)
                for i, (h, qb) in enumerate(phase2[1:]):
                    pend()
                    pend = attn_chunk(1, h, qb, a2a_dst1)
                    if i == 2:
                        out_chunk(af0, 0, 0)
                    elif i == 3:
                        out_chunk(af0, 1, 128)
                pend()
                af1 = a2a_start(*ab1, TPB)
                out_chunk(af1, 0, 256)
                out_chunk(af1, 1, 384)

    nc.finalize()
    return nc


_NC_CACHE = None


def kernel(query_inp, key_inp, value_inp, Wq, bq, Wk, bk, Wv, bv, Wo, bo):
    y, _ = _run(query_inp, key_inp, value_inp, Wq, bq, Wk, bk, Wv, bv, Wo, bo)
    return y


def _bf16(x):
    return np.ascontiguousarray(np.asarray(x, np.float32)).astype(
        ml_dtypes.bfloat16)


def _make_in_maps(query_inp, key_inp, value_inp, Wq, bq, Wk, bk, Wv, bv, Wo, bo):
    xqT = _bf16(np.asarray(query_inp, np.float32).reshape(T, D).T)
    xkT = _bf16(np.asarray(key_inp, np.float32).reshape(T, D).T)
    xvT = _bf16(np.asarray(value_inp, np.float32).reshape(T, D).T)
    Wq = np.asarray(Wq, dtype=np.float32)
    Wk = np.asarray(Wk, dtype=np.float32)
    Wv = np.asarray(Wv, dtype=np.float32)
    Wo = _bf16(Wo)
    in_maps = []
    for c in range(NCORES):
        sl = slice(FPC * c, FPC * (c + 1))
        in_maps.append({
            "xqT": xqT, "xkT": xkT, "xvT": xvT,
            "wq": _bf16(Wq[:, sl]),
            "wk": _bf16(Wk[:, sl]),
            "wv": _bf16(Wv[:, sl]),
            "wo": Wo,
            "bq": np.ascontiguousarray(np.asarray(bq, np.float32)[sl]),
            "bk": np.ascontiguousarray(np.asarray(bk, np.float32)[sl]),
            "bv": np.ascontiguousarray(np.asarray(bv, np.float32)[sl]),
            "bo": _bf16(bo),
        })
    return in_maps


def _assemble(results):
    """y rows per core c: [0,256) = b0 tokens [256c,+256);
    [256,512) = b1 tokens [256c,+256)."""
    y_full = np.empty((T, D), np.float32)
    for c in range(NCORES):
        yc = results[c]["y"]
        y_full[TPB * c: TPB * (c + 1)] = yc[0:TPB]
        y_full[S + TPB * c: S + TPB * (c + 1)] = yc[TPB:]
    return y_full.reshape(B, S, D)


def _run(query_inp, key_inp, value_inp, Wq, bq, Wk, bk, Wv, bv, Wo, bo,
         **run_kwargs):
    global _NC_CACHE
    if _NC_CACHE is None:
        _NC_CACHE = build_nc()
    nc = _NC_CACHE
    in_maps = _make_in_maps(query_inp, key_inp, value_inp,
                            Wq, bq, Wk, bk, Wv, bv, Wo, bo)
    res = run_bass_kernel_spmd(nc, in_maps, core_ids=list(range(NCORES)),
                               **run_kwargs)
    return _assemble(res.results), res


# revision 28
# speedup vs baseline: 4.1428x; 1.0547x over previous
"""Multi-head attention (B=2, S=2048, D=1024, 16 heads) on 8 trn2 cores.

Sharding: tensor-parallel over heads (2 heads = 128 feature dims per core).
v2: all-bf16 data path (host-cast inputs/weights), batched x-tile DMAs
(one [128, 8, 512] load per projection chunk), fused flush (DVE reads the
softmax denominator straight out of PSUM, normalize writes bf16), bf16
AllToAll payloads, direct PSUM->DRAM output stores, and a schedule that
starts the batch-0 collective as early as possible while interleaving two
batch-1 attention chunks into the batch-0 phase to balance the ACT (exp)
engine load across the whole kernel.

Per core, per batch b:
  - Q/K projections computed transposed:  QT/KT [128f, 2048t] = W_c^T @ x^T
  - V projected transposed then PE-transposed back to natural [t, f] layout,
    with a ones-column appended per head (softmax denominator trick)
  - scores computed transposed S^T[k, q] = (KT slice).T @ (QT slice) per head;
    exp on ACT (scale=1/8 folded in) -> bf16; PV matmul lhsT=[V_h|1] gives
    attn^T [64, q] plus the softmax denominator in row 64
  - normalize: DVE copies denom row from PSUM, reciprocal, gpsimd
    partition_broadcast, DVE multiply (PSUM x bcast -> bf16 SBUF)
  - per-batch AllToAll redistributes head-shards -> token-shards
  - output projection over full 1024 features for this core's 256 tokens of b
Host only slices/casts/transposes inputs and re-assembles the token-shard
outputs (core c owns tokens [256c,256c+256) of each batch).
"""

import numpy as np
import ml_dtypes

import concourse.bacc as bacc
import concourse.mybir as mybir
import concourse.tile as tile
from concourse.bass_utils import run_bass_kernel_spmd
from concourse.masks import make_identity

F32 = mybir.dt.float32
BF16 = mybir.dt.bfloat16
AF = mybir.ActivationFunctionType

NCORES = 8
B, S, D = 2, 2048, 1024
T = B * S            # 4096 flattened tokens
HD = 64              # head dim
FPC = 128            # feature dims per core (2 heads)
TPB = S // NCORES    # 256 tokens per core per batch for the output projection
NDT = D // 128       # 8 contraction tiles of 128
QBLK = 512           # q tile (free dim) in attention
NKT = S // 128       # 16 k tiles per (b, h)
KGRP = 2             # k-tiles per exp() activation instruction


def build_nc(repeat=1, use_cc=True):
    nc = bacc.Bacc(trn_type="TRN2", num_devices=NCORES)

    xqT = nc.dram_tensor("xqT", [D, T], BF16, kind="ExternalInput")
    xkT = nc.dram_tensor("xkT", [D, T], BF16, kind="ExternalInput")
    xvT = nc.dram_tensor("xvT", [D, T], BF16, kind="ExternalInput")
    wq = nc.dram_tensor("wq", [D, FPC], BF16, kind="ExternalInput")
    wk = nc.dram_tensor("wk", [D, FPC], BF16, kind="ExternalInput")
    wv = nc.dram_tensor("wv", [D, FPC], BF16, kind="ExternalInput")
    wo = nc.dram_tensor("wo", [D, D], BF16, kind="ExternalInput")
    bq = nc.dram_tensor("bq", [FPC], F32, kind="ExternalInput")
    bk = nc.dram_tensor("bk", [FPC], F32, kind="ExternalInput")
    bv = nc.dram_tensor("bv", [FPC], F32, kind="ExternalInput")
    bo = nc.dram_tensor("bo", [D], BF16, kind="ExternalInput")
    # rows [b*256, b*256+256) = tokens [b*2048+256c, +256) of batch b
    y = nc.dram_tensor("y", [B * TPB, D], F32, kind="ExternalOutput")

    with tile.TileContext(nc) as tc:
        with (
            tc.tile_pool(name="persist", bufs=1) as persist,
            tc.tile_pool(name="dram", bufs=1, space="DRAM") as dram,
            tc.tile_pool(name="xs", bufs=3) as xs_pool,
            tc.tile_pool(name="vt_tmp", bufs=2) as vt_tmp_pool,
            tc.tile_pool(name="es", bufs=6) as es_pool,
            tc.tile_pool(name="den", bufs=4) as den_pool,
            tc.tile_pool(name="bc", bufs=2) as bc_pool,
            tc.tile_pool(name="afl", bufs=2) as afl_pool,
            tc.tile_pool(name="ysb", bufs=2) as ysb_pool,
            tc.tile_pool(name="s_ps", bufs=2, space="PSUM") as s_ps_pool,
            tc.tile_pool(name="uni_ps", bufs=4, space="PSUM") as uni_ps_pool,
        ):
            # ---- resident weights / constants ----
            wq_sb = persist.tile([128, NDT, FPC], BF16)
            wk_sb = persist.tile([128, NDT, FPC], BF16)
            wv_sb = persist.tile([128, NDT, FPC], BF16)
            for w_dram, w_sb in ((wk, wk_sb), (wq, wq_sb), (wv, wv_sb)):
                nc.sync.dma_start(
                    w_sb[:], w_dram.rearrange("(dt p) f -> p dt f", p=128)
                )
            wo_sb = persist.tile([128, NDT, D], BF16)
            bq_sb = persist.tile([128, 1], F32)
            bk_sb = persist.tile([128, 1], F32)
            # bv folded into V at projection time: softmax weights sum to 1
            # after normalization, so (V + bv) then attn == attn then + bv.
            bv_sb = persist.tile([128, 1], F32)
            nc.sync.dma_start(bq_sb[:], bq[:, None])
            nc.sync.dma_start(bk_sb[:], bk[:, None])
            nc.sync.dma_start(bv_sb[:], bv[:, None])
            bo_sb = persist.tile([1, D], BF16)
            nc.sync.dma_start(bo_sb[:], bo[None, :])
            bo_bc = persist.tile([128, D], BF16)
            nc.gpsimd.partition_broadcast(bo_bc[:], bo_sb[:])
            const_f32 = persist.tile([128, 128], F32)
            nc.vector.memset(const_f32[:], 1.0)
            const_bf = persist.tile([128, 128], BF16)
            nc.vector.tensor_copy(const_bf[:], const_f32[:])
            ones_sb = persist.tile([1, 128], BF16)
            nc.vector.tensor_copy(ones_sb[:], const_f32[0:1, :])
            ident_f32 = persist.tile([128, 128], F32)
            make_identity(nc, ident_f32[:])
            ident = persist.tile([128, 128], BF16)
            nc.vector.tensor_copy(ident[:], ident_f32[:])

            # ---- persistent activations ----
            qt_sb = persist.tile([128, T], BF16)   # QT: [feat, tok]
            kt_sb = persist.tile([128, T], BF16)   # KT
            v_sb = persist.tile([128, NKT * B, 130], BF16)  # [k, kt, VA|1|VB|1]
            # per-head normalized attn^T (bf16) for a2a staging
            attn_bf = [persist.tile([64, S], BF16, name=f"attnbf{h}")
                       for h in range(2)]

            # ones columns of V (heads A and B)
            nc.vector.tensor_copy(
                v_sb[:, :, 64:65].rearrange("p a b -> p (a b)"), const_bf[:, 0:32]
            )
            nc.vector.tensor_copy(
                v_sb[:, :, 129:130].rearrange("p a b -> p (a b)"), const_bf[:, 0:32]
            )

            kgrps = []
            k0 = 0
            while k0 < NKT:
                kgrps.append((k0, min(KGRP, NKT - k0)))
                k0 += KGRP

            def proj_chunk(b, kind, tb, split=1):
                """One 512-token t_blk of the K/Q/V projection for batch b."""
                x_dram, w_sb, bias, dst = {
                    "K": (xkT, wk_sb, bk_sb, kt_sb),
                    "Q": (xqT, wq_sb, bq_sb, qt_sb),
                    "V": (xvT, wv_sb, None, None),
                }[kind]
                tlo = b * S + tb * QBLK
                ps = uni_ps_pool.tile([128, QBLK], F32, tag="uni")
                dtg = NDT // split
                for s in range(split):
                    xt = xs_pool.tile([128, dtg, QBLK], BF16, tag="xt")
                    nc.sync.dma_start(
                        xt[:],
                        x_dram[128 * dtg * s:128 * dtg * (s + 1),
                               tlo:tlo + QBLK].rearrange(
                            "(dt p) t -> p dt t", p=128),
                    )
                    for i in range(dtg):
                        dt_i = dtg * s + i
                        nc.tensor.matmul(
                            ps[:], lhsT=w_sb[:, dt_i, :], rhs=xt[:, i, :],
                            start=(dt_i == 0), stop=(dt_i == NDT - 1),
                        )
                if kind != "V":
                    nc.vector.tensor_scalar_add(
                        dst[:, tlo:tlo + QBLK], ps[:], bias[:]
                    )
                    return
                # V: PE-transpose into natural [tok, feat] layout
                # (bv added here, per-partition = per-feature on V^T)
                vt_tmp = vt_tmp_pool.tile([128, QBLK], BF16, tag="vt_tmp")
                nc.vector.tensor_scalar_add(vt_tmp[:], ps[:], bv_sb[:])
                for j in range(QBLK // 128):
                    kt_i = (b * 4 + tb) * 4 + j
                    tp = uni_ps_pool.tile([128, 128], BF16, tag="uni")
                    nc.tensor.transpose(
                        tp[:], vt_tmp[:, 128 * j:128 * (j + 1)], ident[:]
                    )
                    # cols 0:64 -> V 0:64 (head A); 64:128 -> 65:129 (head B)
                    nc.vector.tensor_copy(
                        v_sb[:, kt_i, :].rearrange("p (h x) -> p h x", h=2)[
                            :, :, 0:64
                        ],
                        tp[:].rearrange("p (h x) -> p h x", h=2),
                    )

            def attn_chunk(b, h, qb, a2a_dst, group_feed=None):
                hs = 64 * h
                qlo = b * S + qb * QBLK
                pv = uni_ps_pool.tile([128, QBLK], F32, tag="uni")
                for gi, (k0, klen) in enumerate(kgrps):
                    if group_feed is not None and gi in group_feed:
                        group_feed[gi]()
                    sp = s_ps_pool.tile([128, KGRP * QBLK], F32, tag="sp")
                    for i in range(klen):
                        klo = b * S + (k0 + i) * 128
                        nc.tensor.matmul(
                            sp[:, QBLK * i:QBLK * (i + 1)],
                            lhsT=kt_sb[hs:hs + 64, klo:klo + 128],
                            rhs=qt_sb[hs:hs + 64, qlo:qlo + QBLK],
                            start=True, stop=True,
                        )
                    es = es_pool.tile([128, KGRP * QBLK], BF16, tag="es")
                    nc.scalar.activation(
                        es[:, :QBLK * klen], sp[:, :QBLK * klen],
                        AF.Exp, scale=0.125,
                    )
                    for i in range(klen):
                        kt_i = b * NKT + k0 + i
                        nc.tensor.matmul(
                            pv[0:65, :],
                            lhsT=v_sb[:, kt_i, 65 * h:65 * (h + 1)],
                            rhs=es[:, QBLK * i:QBLK * (i + 1)],
                            start=(k0 + i == 0), stop=(k0 + i == NKT - 1),
                        )

                def flush():
                    qsl = slice(qb * QBLK, (qb + 1) * QBLK)
                    den = den_pool.tile([1, QBLK], mybir.dt.float32r, tag="den")
                    nc.vector.tensor_copy(den[:], pv[64:65, :])
                    with nc.allow_low_precision(reason="softmax denom recip"):
                        nc.vector.reciprocal(den[:], den[:])
                    bcast = bc_pool.tile([64, QBLK], mybir.dt.float32r,
                                         tag="bcast")
                    nc.gpsimd.partition_broadcast(bcast[:], den[:])
                    nc.vector.tensor_tensor(
                        attn_bf[h][:, qsl], pv[0:64, :], bcast[:],
                        mybir.AluOpType.mult,
                    )
                    # stage into the a2a input: nsh shards cover this q-slice
                    buf, sh0, nsh = a2a_dst(qb)
                    nc.sync.dma_start(
                        buf.rearrange("r (p t) -> p r t", p=128)[
                            64 * h:64 * (h + 1), sh0:sh0 + nsh, :
                        ],
                        attn_bf[h][:, qsl].rearrange("p (r t) -> p r t", r=nsh),
                    )

                return flush

            def a2a_start(a2a_in, a2a_out, tw):
                # inputs already staged incrementally by the chunk flushes
                if use_cc:
                    nc.gpsimd.collective_compute(
                        "AllToAll",
                        mybir.AluOpType.bypass,
                        ins=[a2a_in[:]],
                        outs=[a2a_out[:]],
                        replica_groups=[list(range(NCORES))],
                    )
                else:
                    nc.sync.dma_start(a2a_out[:], a2a_in[:])
                afull = afl_pool.tile([128, NCORES, TPB], BF16, tag="afull")
                nc.sync.dma_start(
                    afull[:, :, :tw], a2a_out.rearrange("r (p t) -> p r t", p=128)
                )
                return afull

            def out_chunk(afull, tj, yrow):
                y_sb = ysb_pool.tile([128, D], F32, tag="ysb")
                for n in range(D // QBLK):
                    yp = uni_ps_pool.tile([128, QBLK], F32, tag="uni")
                    for fi in range(NDT):
                        nc.tensor.matmul(
                            yp[:],
                            lhsT=afull[:, fi, 128 * tj:128 * (tj + 1)],
                            rhs=wo_sb[:, fi, QBLK * n:QBLK * (n + 1)],
                            start=(fi == 0), stop=(fi == NDT - 1),
                        )
                    nc.vector.tensor_tensor(
                        y_sb[:, QBLK * n:QBLK * (n + 1)], yp[:],
                        bo_bc[:, QBLK * n:QBLK * (n + 1)], mybir.AluOpType.add,
                    )
                nc.sync.dma_start(y[yrow:yrow + 128, :], y_sb[:])

            def load_wo():
                nc.sync.dma_start(
                    wo_sb[:], wo.rearrange("(ft p) e -> p ft e", p=128)
                )

            for rep in range(repeat):
                # one full-batch a2a (256-token shards) per batch
                ab0 = (dram.tile([NCORES, FPC * TPB], BF16, name=f"a0i_{rep}"),
                       dram.tile([NCORES, FPC * TPB], BF16, name=f"a0o_{rep}"))
                ab1 = (dram.tile([NCORES, FPC * TPB], BF16, name=f"a1i_{rep}"),
                       dram.tile([NCORES, FPC * TPB], BF16, name=f"a1o_{rep}"))

                def a2a_dst0(qb):
                    return ab0[0], 2 * qb, 2

                def a2a_dst1(qb):
                    return ab1[0], 2 * qb, 2

                # batch-0 lead-in: only Q.tb0 + K.tb0; later K and V chunks
                # are emitted inside attn chunk 0 just before the score/PV
                # groups that consume them (group g uses k-tiles 2g, 2g+1;
                # proj chunk tb covers tiles 4tb..4tb+3).
                proj_chunk(0, "Q", 0, split=2)
                proj_chunk(0, "K", 0, split=2)
                def _gf(*specs):
                    def emit():
                        for kind, tb in specs:
                            proj_chunk(0, kind, tb)
                    return emit

                gfeed = {
                    0: _gf(("V", 0), ("K", 1)),
                    1: _gf(("V", 1)),
                    2: _gf(("K", 2)),
                    3: _gf(("V", 2)),
                    4: _gf(("K", 3)),
                    5: _gf(("V", 3)),
                }
                # Q1.tb3 is deferred to phase 2 (fills an ACT-bound gap there)
                feed = (
                    [("Q", 0, tb) for tb in (1, 2, 3)]
                    + [x for tb in range(4) for x in (("K", 1, tb), ("V", 1, tb))]
                    + [("Q", 1, 0)]
                    + [("Q", 1, 1), ("Q", 1, 2)]
                    + ([("WO",)] if rep == 0 else [])
                )
                fi_ = 0

                def emit_feed(n):
                    nonlocal fi_
                    for _ in range(n):
                        if fi_ < len(feed):
                            f = feed[fi_]
                            fi_ += 1
                            if f[0] == "WO":
                                load_wo()
                            else:
                                proj_chunk(f[1], f[0], f[2])

                # phase 1: b0 chunks 0-5, then two early b1 qb0 chunks (their
                # exp load lands in the b0 window), then b0 chunks 6-7.
                plan0 = [(0, h, qb) for qb in range(4) for h in range(2)]
                phase1 = plan0[:6] + [(1, 0, 0), (1, 1, 0)] + plan0[6:]

                pend = attn_chunk(0, *phase1[0][1:], a2a_dst0, group_feed=gfeed)
                for (b, h, qb) in phase1[1:]:
                    emit_feed(2)
                    pend()
                    dst = a2a_dst0 if b == 0 else a2a_dst1
                    pend = attn_chunk(b, h, qb, dst)
                pend()
                emit_feed(len(feed))
                af0 = a2a_start(*ab0, TPB)

                # phase 2: remaining b1 chunks; out chunks for batch 0
                # interleave once af0 is available; one b1 a2a at the end.
                phase2 = [(0, 1), (1, 1), (0, 2), (1, 2), (0, 3), (1, 3)]
                pend = attn_chunk(1, *phase2[0], a2a_dst1)
                for i, (h, qb) in enumerate(phase2[1:]):
                    pend()
                    if i == 0:
                        proj_chunk(1, "Q", 3)
                    pend = attn_chunk(1, h, qb, a2a_dst1)
                    if i == 2:
                        out_chunk(af0, 0, 0)
                    elif i == 3:
                        out_chunk(af0, 1, 128)
                pend()
                af1 = a2a_start(*ab1, TPB)
                out_chunk(af1, 0, 256)
                out_chunk(af1, 1, 384)

    nc.finalize()
    return nc


_NC_CACHE = None


def kernel(query_inp, key_inp, value_inp, Wq, bq, Wk, bk, Wv, bv, Wo, bo):
    y, _ = _run(query_inp, key_inp, value_inp, Wq, bq, Wk, bk, Wv, bv, Wo, bo)
    return y


def _bf16(x):
    return np.ascontiguousarray(np.asarray(x, np.float32)).astype(
        ml_dtypes.bfloat16)


def _make_in_maps(query_inp, key_inp, value_inp, Wq, bq, Wk, bk, Wv, bv, Wo, bo):
    xqT = _bf16(np.asarray(query_inp, np.float32).reshape(T, D).T)
    xkT = _bf16(np.asarray(key_inp, np.float32).reshape(T, D).T)
    xvT = _bf16(np.asarray(value_inp, np.float32).reshape(T, D).T)
    Wq = np.asarray(Wq, dtype=np.float32)
    Wk = np.asarray(Wk, dtype=np.float32)
    Wv = np.asarray(Wv, dtype=np.float32)
    Wo = _bf16(Wo)
    in_maps = []
    for c in range(NCORES):
        sl = slice(FPC * c, FPC * (c + 1))
        in_maps.append({
            "xqT": xqT, "xkT": xkT, "xvT": xvT,
            "wq": _bf16(Wq[:, sl]),
            "wk": _bf16(Wk[:, sl]),
            "wv": _bf16(Wv[:, sl]),
            "wo": Wo,
            "bq": np.ascontiguousarray(np.asarray(bq, np.float32)[sl]),
            "bk": np.ascontiguousarray(np.asarray(bk, np.float32)[sl]),
            "bv": np.ascontiguousarray(np.asarray(bv, np.float32)[sl]),
            "bo": _bf16(bo),
        })
    return in_maps


def _assemble(results):
    """y rows per core c: [0,256) = b0 tokens [256c,+256);
    [256,512) = b1 tokens [256c,+256)."""
    y_full = np.empty((T, D), np.float32)
    for c in range(NCORES):
        yc = results[c]["y"]
        y_full[TPB * c: TPB * (c + 1)] = yc[0:TPB]
        y_full[S + TPB * c: S + TPB * (c + 1)] = yc[TPB:]
    return y_full.reshape(B, S, D)


def _run(query_inp, key_inp, value_inp, Wq, bq, Wk, bk, Wv, bv, Wo, bo,
         **run_kwargs):
    global _NC_CACHE
    if _NC_CACHE is None:
        _NC_CACHE = build_nc()
    nc = _NC_CACHE
    in_maps = _make_in_maps(query_inp, key_inp, value_inp,
                            Wq, bq, Wk, bk, Wv, bv, Wo, bo)
    res = run_bass_kernel_spmd(nc, in_maps, core_ids=list(range(NCORES)),
                               **run_kwargs)
    return _assemble(res.results), res
